# revision 1
# baseline (speedup 1.0000x reference)
"""GQA kernel for Trainium2, 8 NeuronCores.

Sharding: DP=2 over batch x TP=4 over KV-head groups (2 KV heads = 8 query
heads per core).  Each core computes attention for its heads on its batch
element and a partial output projection (wo rows for its heads); the host sums
the 4 TP partials per batch element.

Model dims (hardcoded): bsz=2, seq=2048, dim=2048, 32 q-heads, 8 kv-heads,
head_dim=64.

Per-core dataflow (all fp32 storage; matmuls run as float32r):
  phase 1: PE-transpose x[b] -> XT [d, s]; QT[h,s] = wq_p.T @ XT (row-pair
           layout: partition 0-63 = kv-group a=0, 64-127 = a=1 for the same
           rep index r); K/V projection fused (rhs = [wk|wv], N=256); K
           transposed on PE into KT [hk, s]; V kept natural [s, hv].
  phase 2: per (q-block 512, k-tile 128, r): row-packed score matmuls
           scoresT[k,q] (contraction = head_dim 64, two kv groups packed via
           tile_position rows), exp on ScalarE (scale=1/8 fused; no
           max-subtraction -- scores are O(+-5)), col-packed PV matmuls
           accumulate attnT[hd,q] in PSUM, softmax denominators via
           ones-matmul 4-col-packs into 2 PSUM banks, then normalize at
           eviction (reciprocal + broadcast multiply).
  phase 3: Y_partial = A.T-chunks @ wo_p (row-permuted on host so chunks are
           contiguous), streamed to DRAM.
"""

import os
from contextlib import ExitStack

import numpy as np

BSZ, SEQ, DIM = 2, 2048, 2048
NH, NKV, HD = 32, 8, 64
P = 128
SCALE = 1.0 / 8.0  # 1/sqrt(64)

_CACHE: dict = {}


def build_nc():
    import concourse.bacc as bacc
    import concourse.mybir as mybir
    import concourse.tile as tile
    from concourse.masks import make_identity

    F32 = mybir.dt.float32
    F32R = mybir.dt.float32r
    BF16 = mybir.dt.bfloat16
    EXP = mybir.ActivationFunctionType.Exp
    use_f32r = os.environ.get("GQA_MM_FP32", "0") != "1"
    DTM = F32R if use_f32r else F32  # dtype of every matmul operand tile

    def mm(ap):
        return ap

    nc = bacc.Bacc("TRN2", target_bir_lowering=False, debug=False, num_devices=8)

    x_d = nc.dram_tensor("x", [SEQ, DIM], F32, kind="ExternalInput").ap()
    wq_d = nc.dram_tensor("wq", [DIM, 512], DTM, kind="ExternalInput").ap()
    wkv_d = nc.dram_tensor("wkv", [DIM, 256], DTM, kind="ExternalInput").ap()
    wo_d = nc.dram_tensor("wo", [512, DIM], DTM, kind="ExternalInput").ap()
    y_d = nc.dram_tensor("y", [SEQ, DIM], F32, kind="ExternalOutput").ap()

    with tile.TileContext(nc) as tc, ExitStack() as ctx, \
            nc.allow_low_precision(reason="float32r matmul operands (rounded on write)"):
        persist = ctx.enter_context(tc.tile_pool(name="persist", bufs=1))
        xin_pool = ctx.enter_context(tc.tile_pool(name="xin", bufs=2))
        xt_pool = ctx.enter_context(tc.tile_pool(name="xt", bufs=1))
        pt_pool = ctx.enter_context(tc.tile_pool(name="pt", bufs=16))
        sb_small = ctx.enter_context(tc.tile_pool(name="small", bufs=4))
        ysb_pool = ctx.enter_context(tc.tile_pool(name="ysb", bufs=2))
        ps_a = ctx.enter_context(tc.tile_pool(name="ps_a", bufs=2, space="PSUM"))
        ps_b = ctx.enter_context(tc.tile_pool(name="ps_b", bufs=2, space="PSUM"))
        ps_att = ctx.enter_context(tc.tile_pool(name="ps_att", bufs=4, space="PSUM"))

        # --- persistent tiles -------------------------------------------------
        wq_sb = persist.tile([P, 16, 512], DTM, tag="wq")
        nc.sync.dma_start(out=wq_sb[:], in_=wq_d.rearrange("(t p) h -> p t h", p=P))
        wkv_sb = persist.tile([P, 16, 256], DTM, tag="wkv")
        nc.sync.dma_start(out=wkv_sb[:], in_=wkv_d.rearrange("(t p) h -> p t h", p=P))

        qt_sb = [persist.tile([P, SEQ], DTM, tag=f"qt{r}", name=f"qt{r}") for r in range(4)]
        kt_sb = persist.tile([P, SEQ], DTM, tag="kt")
        v_sb = persist.tile([P, 16, 128], BF16, tag="v")
        # attention output reuses the QT tiles: qt_sb[r][:, q-block] is dead
        # once the q-block's score matmuls are done, and the normalized
        # attnT has exactly the same layout (Tile handles the WAR dep).
        at_sb = qt_sb

        ident = persist.tile([P, P], F32, tag="ident")
        make_identity(nc, ident[:])
        ones64 = persist.tile([P, 64], BF16, tag="ones")
        nc.vector.memset(ones64[:], 1.0)
        ones1 = persist.tile([P, 64], F32, tag="ones1")
        nc.vector.memset(ones1[:], 1.0)

        # --- phase 1: transpose + projections --------------------------------
        for sb in range(4):
            xt = xt_pool.tile([P, 16, 512], DTM, tag="xt")
            for stl in range(4):
                st = sb * 4 + stl
                xin = xin_pool.tile([P, DIM], F32, tag="xin")
                nc.sync.dma_start(out=xin[:], in_=x_d[st * P:(st + 1) * P, :])
                for dg in range(4):
                    tp = ps_a.tile([P, 512], F32, tag="w")
                    for j in range(4):
                        di = dg * 4 + j
                        nc.tensor.matmul(
                            tp[:, j * P:(j + 1) * P],
                            xin[:, di * P:(di + 1) * P],
                            ident[:],
                            is_transpose=True,
                            start=(j == 0),
                            stop=(j == 3),
                        )
                    nc.vector.tensor_copy(
                        xt[:, dg * 4:(dg + 1) * 4, stl * P:(stl + 1) * P],
                        tp[:].rearrange("p (a b) -> p a b", a=4),
                    )
            # QT for this s-block
            for r in range(4):
                qp = ps_b.tile([P, 512], F32, tag="b")
                for di in range(16):
                    nc.tensor.matmul(
                        qp[:],
                        mm(wq_sb[:, di, r * P:(r + 1) * P]),
                        mm(xt[:, di, :]),
                        start=(di == 0),
                        stop=(di == 15),
                    )
                nc.vector.tensor_copy(qt_sb[r][:, sb * 512:(sb + 1) * 512], qp[:])
            # K/V for this s-block
            ktps = ps_att.tile([P, 512], F32, tag="att")
            for stl in range(4):
                st = sb * 4 + stl
                kvp = ps_b.tile([P, 256], F32, tag="b")
                for di in range(16):
                    nc.tensor.matmul(
                        kvp[:],
                        mm(xt[:, di, stl * P:(stl + 1) * P]),
                        mm(wkv_sb[:, di, :]),
                        start=(di == 0),
                        stop=(di == 15),
                    )
                nc.vector.tensor_copy(v_sb[:, st, :], kvp[:, 128:256])
                kt_tmp = sb_small.tile([P, P], F32, tag="ktmp", bufs=2)
                nc.vector.tensor_copy(kt_tmp[:], kvp[:, 0:128])
                nc.tensor.matmul(
                    ktps[:, stl * P:(stl + 1) * P],
                    kt_tmp[:],
                    ident[:],
                    is_transpose=True,
                    start=(stl == 0),
                    stop=(stl == 3),
                )
            nc.vector.tensor_copy(kt_sb[:, sb * 512:(sb + 1) * 512], ktps[:])

        # --- phase 2: attention ----------------------------------------------
        for qi in range(4):
            q0 = qi * 512
            att_ps = [ps_att.tile([P, 512], F32, tag="att", name=f"attps{qi}_{i}") for i in range(4)]
            den_ps = [ps_b.tile([P, 512], F32, tag="b", name=f"denps{qi}_{i}") for i in range(2)]
            for k0 in range(16):
                pts = []
                for r in range(4):
                    s0 = ps_a.tile([P, 512], F32, tag="w")
                    s1 = ps_a.tile([P, 512], F32, tag="w")
                    nc.tensor.matmul(
                        s0[:],
                        mm(kt_sb[0:64, k0 * P:(k0 + 1) * P]),
                        mm(qt_sb[r][0:64, q0:q0 + 512]),
                        start=True, stop=True,
                        tile_position=(0, 0),
                    )
                    nc.tensor.matmul(
                        s1[:],
                        mm(kt_sb[64:128, k0 * P:(k0 + 1) * P]),
                        mm(qt_sb[r][64:128, q0:q0 + 512]),
                        start=True, stop=True,
                        tile_position=(64, 0),
                    )
                    p0 = pt_pool.tile([P, 512], BF16, tag="pt")
                    p1 = pt_pool.tile([P, 512], BF16, tag="pt")
                    nc.scalar.activation(p0[:], s0[:], EXP, scale=SCALE)
                    nc.scalar.activation(p1[:], s1[:], EXP, scale=SCALE)
                    nc.tensor.matmul(
                        att_ps[r][0:64, :],
                        mm(v_sb[:, k0, 0:64]),
                        mm(p0[:]),
                        start=(k0 == 0), stop=(k0 == 15),
                        tile_position=(0, 0), skip_group_check=True,
                    )
                    nc.tensor.matmul(
                        att_ps[r][64:128, :],
                        mm(v_sb[:, k0, 64:128]),
                        mm(p1[:]),
                        start=(k0 == 0), stop=(k0 == 15),
                        tile_position=(0, 64), skip_group_check=True,
                    )
                    pts.append((p0, p1))
                for a in range(2):
                    for r in range(4):
                        nc.tensor.matmul(
                            den_ps[a][r * 32:(r + 1) * 32, :],
                            mm(ones64[:, 0:32]),
                            mm(pts[r][a][:]),
                            start=(k0 == 0), stop=(k0 == 15),
                            tile_position=(0, r * 32), skip_group_check=True,
                        )
            # reciprocal of the 8 denominator rows (one partition each, kept
            # 32-aligned so they can feed K=1 ones-matmuls that replicate them
            # across 64 partitions -- all-on-chip partition broadcast).
            rda = sb_small.tile([P, 512], F32, tag="rdena", bufs=1, name=f"rda{qi}")
            rdb = sb_small.tile([P, 512], F32, tag="rdenb", bufs=1, name=f"rdb{qi}")
            for r in range(4):
                nc.vector.reciprocal(rda[32 * r:32 * r + 1, :],
                                     den_ps[0][32 * r:32 * r + 1, :])
                nc.vector.reciprocal(rdb[32 * r:32 * r + 1, :],
                                     den_ps[1][32 * r:32 * r + 1, :])
            for r in range(4):
                bcr = ps_a.tile([P, 512], F32, tag="w", name=f"bcr{qi}_{r}")
                nc.tensor.matmul(
                    bcr[0:64, :], ones1[32 * r:32 * r + 1, :],
                    rda[32 * r:32 * r + 1, :],
                    start=True, stop=True, tile_position=(32 * r, 0),
                    skip_group_check=True)
                nc.tensor.matmul(
                    bcr[64:128, :], ones1[32 * r:32 * r + 1, :],
                    rdb[32 * r:32 * r + 1, :],
                    start=True, stop=True, tile_position=(32 * r, 64),
                    skip_group_check=True)
                bc = sb_small.tile([P, 512], F32, tag="bc", bufs=2)
                nc.vector.tensor_copy(bc[:], bcr[:])
                nc.vector.tensor_mul(at_sb[r][:, q0:q0 + 512], att_ps[r][:], bc[:])

        # --- phase 3: output projection --------------------------------------
        wo_r = wo_d.rearrange("(r p) n -> p r n", p=P)
        for nb in range(4):
            wo_sb = ysb_pool.tile([P, 4, 512], DTM, tag="wo", name=f"wo{nb}", bufs=2)
            nc.sync.dma_start(out=wo_sb[:], in_=wo_r[:, :, nb * 512:(nb + 1) * 512])
            for st in range(16):
                wp = ps_a.tile([P, 512], F32, tag="w")
                for r in range(4):
                    nc.tensor.matmul(
                        wp[:],
                        mm(at_sb[r][:, st * P:(st + 1) * P]),
                        mm(wo_sb[:, r, :]),
                        start=(r == 0), stop=(r == 3),
                    )
                ysb = ysb_pool.tile([P, 512], F32, tag="y", bufs=3)
                nc.vector.tensor_copy(ysb[:], wp[:])
                nc.sync.dma_start(
                    out=y_d[st * P:(st + 1) * P, nb * 512:(nb + 1) * 512], in_=ysb[:])

    nc.compile()
    return nc


def make_in_maps(x, wq, wk, wv, wo):
    x = np.ascontiguousarray(np.asarray(x, dtype=np.float32))
    wq = np.ascontiguousarray(np.asarray(wq, dtype=np.float32))
    wk = np.ascontiguousarray(np.asarray(wk, dtype=np.float32))
    wv = np.ascontiguousarray(np.asarray(wv, dtype=np.float32))
    wo = np.ascontiguousarray(np.asarray(wo, dtype=np.float32))
    in_maps = []
    for c in range(8):
        b, t = divmod(c, 4)
        g0 = 2 * t
        perm = np.array(
            [(4 * (g0 + a) + r) * 64 + j
             for r in range(4) for a in range(2) for j in range(64)],
            dtype=np.int64,
        )
        in_maps.append({
            "x": x[b],
            "wq": np.ascontiguousarray(wq[:, perm]),
            "wkv": np.ascontiguousarray(np.concatenate(
                [wk[:, g0 * 64:(g0 + 2) * 64], wv[:, g0 * 64:(g0 + 2) * 64]],
                axis=1)),
            "wo": np.ascontiguousarray(wo[perm, :]),
        })
    return in_maps


def kernel(x, wq, wk, wv, wo):
    from concourse.bass_utils import run_bass_kernel_spmd

    if "nc" not in _CACHE:
        _CACHE["nc"] = build_nc()
    nc = _CACHE["nc"]
    in_maps = make_in_maps(x, wq, wk, wv, wo)
    results = run_bass_kernel_spmd(nc, in_maps, list(range(8))).results
    y = np.empty((BSZ, SEQ, DIM), np.float32)
    for b in range(2):
        y[b] = (results[4 * b]["y"] + results[4 * b + 1]["y"]
                + results[4 * b + 2]["y"] + results[4 * b + 3]["y"])
    return y



# revision 26
# speedup vs baseline: 1.6241x; 1.6241x over previous
"""GQA kernel for Trainium2, 8 NeuronCores.

Sharding: DP=2 over batch x TP=4 over KV-head groups (2 KV heads = 8 query
heads per core).  Each core computes attention for its heads on its batch
element and a partial output projection (wo rows for its heads); the host sums
the 4 TP partials per batch element.

Model dims (hardcoded): bsz=2, seq=2048, dim=2048, 32 q-heads, 8 kv-heads,
head_dim=64.

v2 design notes (all matmul operands bf16, fp32 PSUM accumulation):
  - x is transposed on the HOST (numpy) so XT [d, s] DMAs straight into SBUF:
    no PE transpose pass at all.
  - K is projected per s-tile, transposed on PE into KT [hd*2, s]; V kept
    natural [s, hd*2].
  - attention per 512-q block, in two r-pair passes (PSUM: 2 att banks +
    2x[128,1024] score tiles + 2 utility banks = 8 banks exactly):
    scoresT [k,q] via row-packed matmuls for both kv groups into the two
    512-col halves of a [128,1024] PSUM tile; ONE wide exp per (k0, r) on
    the Act engine; PV accumulates att [128(2 groups),512] in PSUM.  PV
    matmuls run one k0 behind the score matmuls so they never wait on a
    just-finished exp.
  - softmax denominators: DVE accumulates den_acc[r] += p (bf16, 2x mode)
    across k0; one all-ones [128,128] stationary matmul per (r, a) yields
    the partition-sum REPLICATED across 128 partitions (no broadcast step);
    reciprocal + two [64,512] muls normalize into at tiles (reusing qt).
  - QT for s-blocks 1..3, K/V for s-blocks 1..3 and the output projection
    for q-blocks 0..2 are woven as "fillers" into the attention loop's PE
    slack; wo(q-block 3) is the tail.
"""

from collections import deque
from contextlib import ExitStack

import numpy as np

BSZ, SEQ, DIM = 2, 2048, 2048
NH, NKV, HD = 32, 8, 64
P = 128
SCALE = 1.0 / 8.0  # 1/sqrt(64)

_CACHE: dict = {}


def build_nc():
    import concourse.bacc as bacc
    import concourse.mybir as mybir
    import concourse.tile as tile
    from concourse.masks import make_identity

    F32 = mybir.dt.float32
    BF16 = mybir.dt.bfloat16
    EXP = mybir.ActivationFunctionType.Exp

    nc = bacc.Bacc("TRN2", target_bir_lowering=False, debug=False, num_devices=8)

    xt_d = nc.dram_tensor("xt", [DIM, SEQ], BF16, kind="ExternalInput").ap()
    wq_d = nc.dram_tensor("wq", [DIM, 512], BF16, kind="ExternalInput").ap()
    wkv_d = nc.dram_tensor("wkv", [DIM, 256], BF16, kind="ExternalInput").ap()
    wo_d = nc.dram_tensor("wo", [512, DIM], BF16, kind="ExternalInput").ap()
    y_d = nc.dram_tensor("y", [SEQ, DIM], F32, kind="ExternalOutput").ap()

    with tile.TileContext(nc) as tc, ExitStack() as ctx, \
            nc.allow_low_precision(reason="bf16 matmul operands, fp32 accumulation"):
        persist = ctx.enter_context(tc.tile_pool(name="persist", bufs=1))
        pt_pool = ctx.enter_context(tc.tile_pool(name="pt", bufs=6))
        rec_pool = ctx.enter_context(tc.tile_pool(name="rec", bufs=2))
        ysb_pool = ctx.enter_context(tc.tile_pool(name="ysb", bufs=8))
        sb_small = ctx.enter_context(tc.tile_pool(name="small", bufs=2))
        ps_att = ctx.enter_context(tc.tile_pool(name="ps_att", bufs=2, space="PSUM"))
        ps_sc = ctx.enter_context(tc.tile_pool(name="ps_sc", bufs=2, space="PSUM"))
        ps_qw = ctx.enter_context(tc.tile_pool(name="ps_qw", bufs=2, space="PSUM"))

        # --- persistent tiles -------------------------------------------------
        # xt for s-block 0 and wkv are split into quarter tiles so compute can
        # start as soon as the first slivers land.
        xt0_t = [persist.tile([P, 4, 512], BF16, tag=f"xt0{h}", name=f"xt0{h}")
                 for h in range(4)]
        xt_t = [None] + [persist.tile([P, 16, 512], BF16, tag=f"xt{sb}", name=f"xt{sb}")
                         for sb in range(1, 4)]

        def xt_slice(sb, di, lo, hi):
            if sb == 0:
                return xt0_t[di // 4][:, di % 4, lo:hi]
            return xt_t[sb][:, di, lo:hi]

        wq_sb = persist.tile([P, 16, 512], BF16, tag="wq")
        wkv_t = [persist.tile([P, 4, 256], BF16, tag=f"wkv{h}", name=f"wkv{h}")
                 for h in range(4)]
        wo_sb = persist.tile([P, 4, 2048], BF16, tag="wo")
        qt_t = [[persist.tile([P, 512], BF16, tag=f"qt{r}_{sb}", name=f"qt{r}_{sb}")
                 for sb in range(4)] for r in range(4)]
        kt_t = [persist.tile([P, 512], BF16, tag=f"kt{sb}", name=f"kt{sb}")
                for sb in range(4)]
        v_t = [persist.tile([P, 4, 128], BF16, tag=f"v{sb}", name=f"v{sb}")
               for sb in range(4)]
        den_acc = [persist.tile([P, 1024], BF16, tag=f"den{r}", name=f"den{r}")
                   for r in range(4)]
        ones = persist.tile([P, P], BF16, tag="ones")
        ident = persist.tile([P, P], F32, tag="ident")

        # --- input DMAs (ordered: first-needed first) -------------------------
        xt_r = xt_d.rearrange("(t p) s -> p t s", p=P)
        wkv_r = wkv_d.rearrange("(t p) h -> p t h", p=P)
        for h in range(4):
            nc.sync.dma_start(out=wkv_t[h][:], in_=wkv_r[:, 4 * h:4 * h + 4, :])
            nc.sync.dma_start(out=xt0_t[h][:], in_=xt_r[:, 4 * h:4 * h + 4, 0:512])
        nc.sync.dma_start(out=wq_sb[:], in_=wq_d.rearrange("(t p) h -> p t h", p=P))
        for sb in range(1, 4):
            nc.sync.dma_start(out=xt_t[sb][:], in_=xt_r[:, :, sb * 512:(sb + 1) * 512])
        nc.sync.dma_start(out=wo_sb[:], in_=wo_d.rearrange("(r p) n -> p r n", p=P))

        nc.vector.memset(ones[:], 1.0)
        make_identity(nc, ident[:])

        # --- work chunks ------------------------------------------------------
        # The KT transpose of a K/V chunk is deferred to the next chunk so the
        # PE never waits on the DVE kt_tmp eviction it feeds from.
        pending = []

        def flush_pending():
            while pending:
                pending.pop(0)()

        def kv_chunk(sb, stl):
            """K/V projection for s-tile sb*4+stl; K transposed into kt_t."""
            flush_pending()
            kvp = ps_qw.tile([P, 512], F32, tag="qw")
            for di in range(16):
                nc.tensor.matmul(
                    kvp[:, 0:256],
                    xt_slice(sb, di, stl * P, (stl + 1) * P),
                    wkv_t[di // 4][:, di % 4, :],
                    start=(di == 0),
                    stop=(di == 15),
                )
            nc.vector.tensor_copy(v_t[sb][:, stl, :], kvp[:, 128:256])
            kt_tmp = sb_small.tile([P, P], F32, tag="ktmp")
            nc.vector.tensor_copy(kt_tmp[:], kvp[:, 0:128])

            def do_kt(sb=sb, stl=stl, kt_tmp=kt_tmp):
                ktt = ps_qw.tile([P, 512], F32, tag="qw")
                nc.tensor.matmul(ktt[:, 0:P], kt_tmp[:], ident[:],
                                 is_transpose=True, start=True, stop=True)
                nc.vector.tensor_copy(kt_t[sb][:, stl * P:(stl + 1) * P], ktt[:, 0:P])
            pending.append(do_kt)

        def qt_chunk(sb, r):
            """QT[r] for s-block sb."""
            flush_pending()
            qp = ps_qw.tile([P, 512], F32, tag="qw")
            for di in range(16):
                nc.tensor.matmul(
                    qp[:],
                    wq_sb[:, di, r * P:(r + 1) * P],
                    xt_slice(sb, di, 0, 512),
                    start=(di == 0),
                    stop=(di == 15),
                )
            nc.vector.tensor_copy(qt_t[r][sb][:], qp[:])

        ychunk_n = [0]

        def wo_chunk(st, nb, tail=False):
            """Output projection for s-tile st, output-column block nb."""
            flush_pending()
            if tail and ychunk_n[0] % 2 == 0:
                # the attention banks are dead at the tail: alternating pools
                # doubles the effective wp rotation depth
                wp = ps_att.tile([P, 512], F32, tag="att")
            else:
                wp = ps_qw.tile([P, 512], F32, tag="qw")
            ychunk_n[0] += 1
            stl = st % 4
            for r in range(4):
                nc.tensor.matmul(
                    wp[:],
                    qt_t[r][st // 4][:, stl * P:(stl + 1) * P],
                    wo_sb[:, r, nb * 512:(nb + 1) * 512],
                    start=(r == 0),
                    stop=(r == 3),
                )
            ysb = ysb_pool.tile([P, 512], F32, tag="y")
            # alternate eviction engine (GPSIMD cannot read PSUM) so neither
            # DVE nor Act becomes the bottleneck in the drain
            if ychunk_n[0] % 2 == 0:
                nc.scalar.copy(ysb[:], wp[:])
            else:
                nc.vector.tensor_copy(ysb[:], wp[:])
            nc.sync.dma_start(
                out=y_d[st * P:(st + 1) * P, nb * 512:(nb + 1) * 512], in_=ysb[:])

        # --- head: K/V + QT for s-block 0 ------------------------------------
        for stl in range(4):
            kv_chunk(0, stl)
        for r in range(4):
            qt_chunk(0, r)

        # fillers woven into the attention loop's PE slack.  urgent (KV/QT,
        # intra-attention deadlines) drain one per k0 slot; lazy (wo) one per
        # two slots so late passes stay PE-fed.
        urgent = deque()
        lazy = deque()
        for sb in range(1, 4):
            for stl in range(4):
                urgent.append(lambda sb=sb, stl=stl: kv_chunk(sb, stl))
        for sb in range(1, 4):
            for r in range(4):
                urgent.append(lambda sb=sb, r=r: qt_chunk(sb, r))

        # --- attention --------------------------------------------------------
        for qi in range(4):
            for half in range(2):
                rpair = (2 * half, 2 * half + 1)
                last_pass = (qi == 3 and half == 1)
                att = {r: ps_att.tile([P, 512], F32, tag="att", name=f"att{qi}_{r}")
                       for r in rpair}

                def pv_and_den(k0, pk, skip_den=False):
                    for r, p in pk:
                        nc.tensor.matmul(
                            att[r][0:64, :],
                            v_t[k0 // 4][:, k0 % 4, 0:64],
                            p[:, 0:512],
                            start=(k0 == 0), stop=(k0 == 15),
                            tile_position=(0, 0), skip_group_check=True,
                        )
                        nc.tensor.matmul(
                            att[r][64:128, :],
                            v_t[k0 // 4][:, k0 % 4, 64:128],
                            p[:, 512:1024],
                            start=(k0 == 0), stop=(k0 == 15),
                            tile_position=(0, 64), skip_group_check=True,
                        )
                        if skip_den:
                            continue
                        if k0 == 0:
                            nc.vector.tensor_copy(den_acc[r][:], p[:])
                        else:
                            nc.vector.tensor_add(den_acc[r][:], den_acc[r][:], p[:])

                prev = None
                for k0 in range(16):
                    cur = []
                    for r in rpair:
                        sc = ps_sc.tile([P, 1024], F32, tag="sc")
                        nc.tensor.matmul(
                            sc[:, 0:512],
                            kt_t[k0 // 4][0:64, (k0 % 4) * P:(k0 % 4 + 1) * P],
                            qt_t[r][qi][0:64, :],
                            start=True, stop=True,
                            tile_position=(0, 0),
                        )
                        nc.tensor.matmul(
                            sc[:, 512:1024],
                            kt_t[k0 // 4][64:128, (k0 % 4) * P:(k0 % 4 + 1) * P],
                            qt_t[r][qi][64:128, :],
                            start=True, stop=True,
                            tile_position=(64, 0),
                        )
                        p = pt_pool.tile([P, 1024], BF16, tag="p")
                        nc.scalar.activation(p[:], sc[:], EXP, scale=SCALE)
                        cur.append((r, p))
                    if prev is not None:
                        pv_and_den(k0 - 1, prev)
                    if urgent:
                        urgent.popleft()()
                    elif lazy and k0 % 2 == 0 and (qi < 3 or len(lazy) > 4):
                        lazy.popleft()()
                    prev = cur
                # on the very last pass, keep p(k0=15) out of the DVE den
                # chain: it is accumulated straight into den_all by a second
                # ones-matmul, shortening the exp->den->recip->norm->wo tail.
                pv_and_den(15, prev, skip_den=last_pass)
                p15 = dict(prev)
                flush_pending()

                # pass epilogue: denominators, reciprocal, normalize.  den
                # halves live in the qw pool (not sc) so the next pass's
                # score matmuls never wait on the reciprocal to free a bank.
                for r in rpair:
                    rec = rec_pool.tile([P, 1024], F32, tag="rec")
                    for a in range(2):
                        den_h = ps_qw.tile([P, 512], F32, tag="qw",
                                           name=f"dall{qi}_{r}_{a}")
                        nc.tensor.matmul(den_h[:], ones[:],
                                         den_acc[r][:, a * 512:(a + 1) * 512],
                                         start=True, stop=not last_pass)
                        if last_pass:
                            nc.tensor.matmul(den_h[:], ones[:],
                                             p15[r][:, a * 512:(a + 1) * 512],
                                             start=False, stop=True)
                        nc.vector.reciprocal(rec[:, a * 512:(a + 1) * 512], den_h[:])
                    nc.vector.tensor_mul(qt_t[r][qi][0:64, :], att[r][0:64, :],
                                         rec[0:64, 0:512])
                    nc.vector.tensor_mul(qt_t[r][qi][64:128, :], att[r][64:128, :],
                                         rec[64:128, 512:1024])
                # bridge the epilogue chain with independent PE work so the
                # next pass (or the wo tail) never sees a cold PE
                for _ in range(4 if last_pass else 2):
                    if lazy:
                        lazy.popleft()()
            # wo for this q-block becomes filler work for later q-blocks
            # (except the last q-block, which drains at the tail)
            if qi < 3:
                for nb in range(4):
                    for stl in range(4):
                        lazy.append(lambda st=qi * 4 + stl, nb=nb: wo_chunk(st, nb))

        while urgent:
            urgent.popleft()()
        while lazy:
            lazy.popleft()()
        for nb in range(4):
            for stl in range(4):
                wo_chunk(12 + stl, nb, tail=True)

    nc.compile()
    return nc


def make_in_maps(x, wq, wk, wv, wo):
    import ml_dtypes
    BF = ml_dtypes.bfloat16

    x = np.asarray(x, dtype=np.float32)
    wq = np.asarray(wq, dtype=np.float32)
    wk = np.asarray(wk, dtype=np.float32)
    wv = np.asarray(wv, dtype=np.float32)
    wo = np.asarray(wo, dtype=np.float32)
    in_maps = []
    for c in range(8):
        b, t = divmod(c, 4)
        g0 = 2 * t
        perm = np.array(
            [(4 * (g0 + a) + r) * 64 + j
             for r in range(4) for a in range(2) for j in range(64)],
            dtype=np.int64,
        )
        in_maps.append({
            "xt": np.ascontiguousarray(x[b].T).astype(BF),
            "wq": np.ascontiguousarray(wq[:, perm]).astype(BF),
            "wkv": np.ascontiguousarray(np.concatenate(
                [wk[:, g0 * 64:(g0 + 2) * 64], wv[:, g0 * 64:(g0 + 2) * 64]],
                axis=1)).astype(BF),
            "wo": np.ascontiguousarray(wo[perm, :]).astype(BF),
        })
    return in_maps


def kernel(x, wq, wk, wv, wo):
    from concourse.bass_utils import run_bass_kernel_spmd

    if "nc" not in _CACHE:
        _CACHE["nc"] = build_nc()
    nc = _CACHE["nc"]
    in_maps = make_in_maps(x, wq, wk, wv, wo)
    results = run_bass_kernel_spmd(nc, in_maps, list(range(8))).results
    y = np.empty((BSZ, SEQ, DIM), np.float32)
    for b in range(2):
        y[b] = (results[4 * b]["y"] + results[4 * b + 1]["y"]
                + results[4 * b + 2]["y"] + results[4 * b + 3]["y"])
    return y


# revision 38
# speedup vs baseline: 1.6406x; 1.0101x over previous
"""GQA kernel for Trainium2, 8 NeuronCores.

Sharding: DP=2 over batch x TP=4 over KV-head groups (2 KV heads = 8 query
heads per core).  Each core computes attention for its heads on its batch
element and a partial output projection (wo rows for its heads); the host sums
the 4 TP partials per batch element.

Model dims (hardcoded): bsz=2, seq=2048, dim=2048, 32 q-heads, 8 kv-heads,
head_dim=64.

v2 design notes (all matmul operands bf16, fp32 PSUM accumulation):
  - x is transposed on the HOST (numpy) so XT [d, s] DMAs straight into SBUF:
    no PE transpose pass at all.
  - K is projected per s-tile, transposed on PE into KT [hd*2, s]; V kept
    natural [s, hd*2].
  - attention per 512-q block, in two r-pair passes (PSUM: 2 att banks +
    2x[128,1024] score tiles + 2 utility banks = 8 banks exactly):
    scoresT [k,q] via row-packed matmuls for both kv groups into the two
    512-col halves of a [128,1024] PSUM tile; ONE wide exp per (k0, r) on
    the Act engine; PV accumulates att [128(2 groups),512] in PSUM.  PV
    matmuls run one k0 behind the score matmuls so they never wait on a
    just-finished exp.
  - softmax denominators: DVE accumulates den_acc[r] += p (bf16, 2x mode)
    across k0; one all-ones [128,128] stationary matmul per (r, a) yields
    the partition-sum REPLICATED across 128 partitions (no broadcast step);
    reciprocal + two [64,512] muls normalize into at tiles (reusing qt).
  - QT for s-blocks 1..3, K/V for s-blocks 1..3 and the output projection
    for q-blocks 0..2 are woven as "fillers" into the attention loop's PE
    slack; wo(q-block 3) is the tail.
"""

from collections import deque
from contextlib import ExitStack

import numpy as np

BSZ, SEQ, DIM = 2, 2048, 2048
NH, NKV, HD = 32, 8, 64
P = 128
SCALE = 1.0 / 8.0  # 1/sqrt(64)

_CACHE: dict = {}


def build_nc():
    import concourse.bacc as bacc
    import concourse.mybir as mybir
    import concourse.tile as tile
    from concourse import bass_isa
    from concourse.masks import make_identity

    F32 = mybir.dt.float32
    BF16 = mybir.dt.bfloat16
    EXP = mybir.ActivationFunctionType.Exp

    nc = bacc.Bacc("TRN2", target_bir_lowering=False, debug=False, num_devices=8)

    xt_d = nc.dram_tensor("xt", [DIM, SEQ], BF16, kind="ExternalInput").ap()
    wq_d = nc.dram_tensor("wq", [DIM, 512], BF16, kind="ExternalInput").ap()
    wkv_d = nc.dram_tensor("wkv", [DIM, 256], BF16, kind="ExternalInput").ap()
    wo_d = nc.dram_tensor("wo", [512, DIM], BF16, kind="ExternalInput").ap()
    y_d = nc.dram_tensor("y", [SEQ, DIM], F32, kind="ExternalOutput").ap()

    with tile.TileContext(nc) as tc, ExitStack() as ctx, \
            nc.allow_low_precision(reason="bf16 matmul operands, fp32 accumulation"):
        persist = ctx.enter_context(tc.tile_pool(name="persist", bufs=1))
        pt_pool = ctx.enter_context(tc.tile_pool(name="pt", bufs=6))
        rec_pool = ctx.enter_context(tc.tile_pool(name="rec", bufs=2))
        ysb_pool = ctx.enter_context(tc.tile_pool(name="ysb", bufs=8))
        sb_small = ctx.enter_context(tc.tile_pool(name="small", bufs=2))
        ps_att = ctx.enter_context(tc.tile_pool(name="ps_att", bufs=2, space="PSUM"))
        ps_sc = ctx.enter_context(tc.tile_pool(name="ps_sc", bufs=2, space="PSUM"))
        ps_qw = ctx.enter_context(tc.tile_pool(name="ps_qw", bufs=2, space="PSUM"))

        # --- persistent tiles -------------------------------------------------
        # xt for s-block 0 and wkv are split into quarter tiles so compute can
        # start as soon as the first slivers land.
        xt0_t = [persist.tile([P, 4, 512], BF16, tag=f"xt0{h}", name=f"xt0{h}")
                 for h in range(4)]
        xt_t = [None] + [persist.tile([P, 16, 512], BF16, tag=f"xt{sb}", name=f"xt{sb}")
                         for sb in range(1, 4)]

        def xt_slice(sb, di, lo, hi):
            if sb == 0:
                return xt0_t[di // 4][:, di % 4, lo:hi]
            return xt_t[sb][:, di, lo:hi]

        wq_sb = persist.tile([P, 16, 512], BF16, tag="wq")
        wkv_t = [persist.tile([P, 4, 256], BF16, tag=f"wkv{h}", name=f"wkv{h}")
                 for h in range(4)]
        wo_sb = persist.tile([P, 4, 2048], BF16, tag="wo")
        qt_t = [[persist.tile([P, 512], BF16, tag=f"qt{r}_{sb}", name=f"qt{r}_{sb}")
                 for sb in range(4)] for r in range(4)]
        kt_t = [persist.tile([P, 512], BF16, tag=f"kt{sb}", name=f"kt{sb}")
                for sb in range(4)]
        v_t = [persist.tile([P, 4, 128], BF16, tag=f"v{sb}", name=f"v{sb}")
               for sb in range(4)]
        den_acc = [persist.tile([P, 1024], BF16, tag=f"den{r}", name=f"den{r}")
                   for r in range(4)]
        ones = persist.tile([P, P], BF16, tag="ones")
        ident = persist.tile([P, P], F32, tag="ident")

        # --- input DMAs (ordered: first-needed first) -------------------------
        xt_r = xt_d.rearrange("(t p) s -> p t s", p=P)
        wkv_r = wkv_d.rearrange("(t p) h -> p t h", p=P)
        for h in range(4):
            nc.sync.dma_start(out=wkv_t[h][:], in_=wkv_r[:, 4 * h:4 * h + 4, :])
            nc.sync.dma_start(out=xt0_t[h][:], in_=xt_r[:, 4 * h:4 * h + 4, 0:512])
        nc.sync.dma_start(out=wq_sb[:], in_=wq_d.rearrange("(t p) h -> p t h", p=P))
        for sb in range(1, 4):
            nc.sync.dma_start(out=xt_t[sb][:], in_=xt_r[:, :, sb * 512:(sb + 1) * 512])
        nc.sync.dma_start(out=wo_sb[:], in_=wo_d.rearrange("(r p) n -> p r n", p=P))

        nc.vector.memset(ones[:], 1.0)
        make_identity(nc, ident[:])

        # warm the PE p-state while the first DMAs are in flight: the ramp to
        # full clock needs ~3us of continuous execution, so burn it on dummy
        # transposes instead of the first real matmuls
        warm = ps_att.tile([P, 512], F32, tag="att")
        for i in range(40):
            nc.tensor.matmul(warm[:, 0:P], ident[:], ident[:],
                             is_transpose=True, start=(i == 0), stop=(i == 39))

        # --- work chunks ------------------------------------------------------
        # The KT transpose of a K/V chunk is deferred to the next chunk so the
        # PE never waits on the DVE kt_tmp eviction it feeds from.
        pending = []

        def flush_pending():
            while pending:
                pending.pop(0)()

        def kv_chunk(sb, stl):
            """K/V projection for s-tile sb*4+stl; K transposed into kt_t."""
            flush_pending()
            kvp = ps_qw.tile([P, 512], F32, tag="qw")
            for di in range(16):
                nc.tensor.matmul(
                    kvp[:, 0:256],
                    xt_slice(sb, di, stl * P, (stl + 1) * P),
                    wkv_t[di // 4][:, di % 4, :],
                    start=(di == 0),
                    stop=(di == 15),
                )
            nc.vector.tensor_copy(v_t[sb][:, stl, :], kvp[:, 128:256])
            kt_tmp = sb_small.tile([P, P], F32, tag="ktmp")
            nc.vector.tensor_copy(kt_tmp[:], kvp[:, 0:128])

            def do_kt(sb=sb, stl=stl, kt_tmp=kt_tmp):
                ktt = ps_qw.tile([P, 512], F32, tag="qw")
                nc.tensor.matmul(ktt[:, 0:P], kt_tmp[:], ident[:],
                                 is_transpose=True, start=True, stop=True)
                nc.vector.tensor_copy(kt_t[sb][:, stl * P:(stl + 1) * P], ktt[:, 0:P])
            pending.append(do_kt)

        def qt_chunk(sb, r):
            """QT[r] for s-block sb."""
            flush_pending()
            qp = ps_qw.tile([P, 512], F32, tag="qw")
            for di in range(16):
                nc.tensor.matmul(
                    qp[:],
                    wq_sb[:, di, r * P:(r + 1) * P],
                    xt_slice(sb, di, 0, 512),
                    start=(di == 0),
                    stop=(di == 15),
                )
            nc.vector.tensor_copy(qt_t[r][sb][:], qp[:])

        ychunk_n = [0]

        def wo_chunk(st, nb, tail=False):
            """Output projection for s-tile st, output-column block nb."""
            flush_pending()
            if tail and ychunk_n[0] % 2 == 0:
                # the attention banks are dead at the tail: alternating pools
                # doubles the effective wp rotation depth
                wp = ps_att.tile([P, 512], F32, tag="att")
            else:
                wp = ps_qw.tile([P, 512], F32, tag="qw")
            ychunk_n[0] += 1
            stl = st % 4
            for r in range(4):
                nc.tensor.matmul(
                    wp[:],
                    qt_t[r][st // 4][:, stl * P:(stl + 1) * P],
                    wo_sb[:, r, nb * 512:(nb + 1) * 512],
                    start=(r == 0),
                    stop=(r == 3),
                )
            ysb = ysb_pool.tile([P, 512], F32, tag="y")
            # mid-stream evictions go to DVE (Act is exp-saturated there);
            # tail evictions go to Act, which is idle by then
            if tail:
                nc.scalar.copy(ysb[:], wp[:])
            else:
                nc.vector.tensor_copy(ysb[:], wp[:])
            nc.sync.dma_start(
                out=y_d[st * P:(st + 1) * P, nb * 512:(nb + 1) * 512], in_=ysb[:])

        # --- head: K/V for s-block 0 + QT for the first r-pair ----------------
        for stl in range(4):
            kv_chunk(0, stl)
        for r in range(2):
            qt_chunk(0, r)

        # fillers woven into the attention loop's PE slack.  urgent (KV/QT,
        # intra-attention deadlines) drain one per k0 slot; lazy (wo) one per
        # two slots so late passes stay PE-fed.  QT0 r2/r3 lead (needed by
        # qi0's second pass).
        urgent = deque()
        lazy = deque()
        for r in range(2, 4):
            urgent.append(lambda r=r: qt_chunk(0, r))
        for sb in range(1, 4):
            for stl in range(4):
                urgent.append(lambda sb=sb, stl=stl: kv_chunk(sb, stl))
        for sb in range(1, 4):
            for r in range(4):
                urgent.append(lambda sb=sb, r=r: qt_chunk(sb, r))

        # --- attention --------------------------------------------------------
        for qi in range(4):
            for half in range(2):
                rpair = (2 * half, 2 * half + 1)
                last_pass = (qi == 3 and half == 1)
                att = {r: ps_att.tile([P, 512], F32, tag="att", name=f"att{qi}_{r}")
                       for r in rpair}

                def pv_and_den(k0, pk, skip_den=False):
                    for r, p in pk:
                        nc.tensor.matmul(
                            att[r][0:64, :],
                            v_t[k0 // 4][:, k0 % 4, 0:64],
                            p[:, 0:512],
                            start=(k0 == 0), stop=(k0 == 15),
                            tile_position=(0, 0), skip_group_check=True,
                        )
                        nc.tensor.matmul(
                            att[r][64:128, :],
                            v_t[k0 // 4][:, k0 % 4, 64:128],
                            p[:, 512:1024],
                            start=(k0 == 0), stop=(k0 == 15),
                            tile_position=(0, 64), skip_group_check=True,
                        )
                        if skip_den:
                            continue
                        if k0 == 0:
                            nc.vector.tensor_copy(den_acc[r][:], p[:])
                        else:
                            nc.vector.tensor_add(den_acc[r][:], den_acc[r][:], p[:])

                prev = None
                for k0 in range(16):
                    cur = []
                    for r in rpair:
                        sc = ps_sc.tile([P, 1024], F32, tag="sc")
                        nc.tensor.matmul(
                            sc[:, 0:512],
                            kt_t[k0 // 4][0:64, (k0 % 4) * P:(k0 % 4 + 1) * P],
                            qt_t[r][qi][0:64, :],
                            start=True, stop=True,
                            tile_position=(0, 0),
                        )
                        nc.tensor.matmul(
                            sc[:, 512:1024],
                            kt_t[k0 // 4][64:128, (k0 % 4) * P:(k0 % 4 + 1) * P],
                            qt_t[r][qi][64:128, :],
                            start=True, stop=True,
                            tile_position=(64, 0),
                        )
                        p = pt_pool.tile([P, 1024], BF16, tag="p")
                        nc.scalar.activation(p[:], sc[:], EXP, scale=SCALE)
                        cur.append((r, p))
                    if prev is not None:
                        pv_and_den(k0 - 1, prev)
                    if urgent:
                        urgent.popleft()()
                    elif lazy and k0 % 2 == 0 and (qi < 3 or len(lazy) > 8):
                        lazy.popleft()()
                    prev = cur
                # pass epilogue.  p(k0=15) stays out of the DVE den chain:
                # it is accumulated straight into den_h by a second
                # ones-matmul.  Emission order interleaves the den_acc
                # matmuls (no exp15 dependency) with the deferred PV(15) so
                # the PE never sits on the just-finished exp.
                p15 = dict(prev)
                flush_pending()
                den_hs = {}
                for r in rpair:
                    for a in range(2):
                        den_h = ps_qw.tile([P, 512], F32, tag="qw",
                                           name=f"dall{qi}_{r}_{a}")
                        nc.tensor.matmul(den_h[:], ones[:],
                                         den_acc[r][:, a * 512:(a + 1) * 512],
                                         start=True, stop=False)
                        den_hs[(r, a)] = den_h
                pv_and_den(15, prev, skip_den=True)
                for r in rpair:
                    rec = rec_pool.tile([P, 1024], F32, tag="rec")
                    for a in range(2):
                        nc.tensor.matmul(den_hs[(r, a)][:], ones[:],
                                         p15[r][:, a * 512:(a + 1) * 512],
                                         start=False, stop=True)
                        nc.vector.reciprocal(rec[:, a * 512:(a + 1) * 512],
                                             den_hs[(r, a)][:])
                    nc.vector.tensor_mul(qt_t[r][qi][0:64, :], att[r][0:64, :],
                                         rec[0:64, 0:512])
                    nc.vector.tensor_mul(qt_t[r][qi][64:128, :], att[r][64:128, :],
                                         rec[64:128, 512:1024])
                # bridge the epilogue chain with independent PE work so the
                # next pass (or the wo tail) never sees a cold PE
                for _ in range(8 if last_pass else 2):
                    if lazy:
                        lazy.popleft()()
            # wo for this q-block becomes filler work for later q-blocks
            # (except the last q-block, which drains at the tail)
            if qi < 3:
                for nb in range(4):
                    for stl in range(4):
                        lazy.append(lambda st=qi * 4 + stl, nb=nb: wo_chunk(st, nb))

        while urgent:
            urgent.popleft()()
        while lazy:
            lazy.popleft()()
        for nb in range(4):
            for stl in range(4):
                wo_chunk(12 + stl, nb, tail=True)

    nc.compile()
    return nc


def make_in_maps(x, wq, wk, wv, wo):
    import ml_dtypes
    BF = ml_dtypes.bfloat16

    x = np.asarray(x, dtype=np.float32)
    wq = np.asarray(wq, dtype=np.float32)
    wk = np.asarray(wk, dtype=np.float32)
    wv = np.asarray(wv, dtype=np.float32)
    wo = np.asarray(wo, dtype=np.float32)
    in_maps = []
    for c in range(8):
        b, t = divmod(c, 4)
        g0 = 2 * t
        perm = np.array(
            [(4 * (g0 + a) + r) * 64 + j
             for r in range(4) for a in range(2) for j in range(64)],
            dtype=np.int64,
        )
        in_maps.append({
            "xt": np.ascontiguousarray(x[b].T).astype(BF),
            "wq": np.ascontiguousarray(wq[:, perm]).astype(BF),
            "wkv": np.ascontiguousarray(np.concatenate(
                [wk[:, g0 * 64:(g0 + 2) * 64], wv[:, g0 * 64:(g0 + 2) * 64]],
                axis=1)).astype(BF),
            "wo": np.ascontiguousarray(wo[perm, :]).astype(BF),
        })
    return in_maps


def kernel(x, wq, wk, wv, wo):
    from concourse.bass_utils import run_bass_kernel_spmd

    if "nc" not in _CACHE:
        _CACHE["nc"] = build_nc()
    nc = _CACHE["nc"]
    in_maps = make_in_maps(x, wq, wk, wv, wo)
    results = run_bass_kernel_spmd(nc, in_maps, list(range(8))).results
    y = np.empty((BSZ, SEQ, DIM), np.float32)
    for b in range(2):
        y[b] = (results[4 * b]["y"] + results[4 * b + 1]["y"]
                + results[4 * b + 2]["y"] + results[4 * b + 3]["y"])
    return y


# revision 53
# speedup vs baseline: 1.6689x; 1.0173x over previous
"""GQA kernel for Trainium2, 8 NeuronCores.

Sharding: DP=2 over batch x TP=4 over KV-head groups (2 KV heads = 8 query
heads per core).  Each core computes attention for its heads on its batch
element and a partial output projection (wo rows for its heads); the host sums
the 4 TP partials per batch element.

Model dims (hardcoded): bsz=2, seq=2048, dim=2048, 32 q-heads, 8 kv-heads,
head_dim=64.

v2 design notes (all matmul operands bf16, fp32 PSUM accumulation):
  - x is transposed on the HOST (numpy) so XT [d, s] DMAs straight into SBUF:
    no PE transpose pass at all.
  - K is projected per s-tile, transposed on PE into KT [hd*2, s]; V kept
    natural [s, hd*2].
  - attention per 512-q block, in two r-pair passes (PSUM: 2 att banks +
    2x[128,1024] score tiles + 2 utility banks = 8 banks exactly):
    scoresT [k,q] via row-packed matmuls for both kv groups into the two
    512-col halves of a [128,1024] PSUM tile; ONE wide exp per (k0, r) on
    the Act engine; PV accumulates att [128(2 groups),512] in PSUM.  PV
    matmuls run one k0 behind the score matmuls so they never wait on a
    just-finished exp.
  - softmax denominators: DVE accumulates den_acc[r] += p (bf16, 2x mode)
    across k0; one all-ones [128,128] stationary matmul per (r, a) yields
    the partition-sum REPLICATED across 128 partitions (no broadcast step);
    reciprocal + two [64,512] muls normalize into at tiles (reusing qt).
  - QT for s-blocks 1..3, K/V for s-blocks 1..3 and the output projection
    for q-blocks 0..2 are woven as "fillers" into the attention loop's PE
    slack; wo(q-block 3) is the tail.
"""

from collections import deque
from contextlib import ExitStack

import numpy as np

BSZ, SEQ, DIM = 2, 2048, 2048
NH, NKV, HD = 32, 8, 64
P = 128
SCALE = 1.0 / 8.0  # 1/sqrt(64)

_CACHE: dict = {}


def build_nc():
    import concourse.bacc as bacc
    import concourse.mybir as mybir
    import concourse.tile as tile
    from concourse import bass_isa
    F32 = mybir.dt.float32
    BF16 = mybir.dt.bfloat16
    EXP = mybir.ActivationFunctionType.Exp

    nc = bacc.Bacc("TRN2", target_bir_lowering=False, debug=False, num_devices=8)

    xt_d = nc.dram_tensor("xt", [DIM, SEQ], BF16, kind="ExternalInput").ap()
    wq_d = nc.dram_tensor("wq", [DIM, 512], BF16, kind="ExternalInput").ap()
    wkv_d = nc.dram_tensor("wkv", [DIM, 256], BF16, kind="ExternalInput").ap()
    wo_d = nc.dram_tensor("wo", [512, DIM], BF16, kind="ExternalInput").ap()
    y_d = nc.dram_tensor("y", [SEQ, DIM], F32, kind="ExternalOutput").ap()

    with tile.TileContext(nc) as tc, ExitStack() as ctx, \
            nc.allow_low_precision(reason="bf16 matmul operands, fp32 accumulation"):
        persist = ctx.enter_context(tc.tile_pool(name="persist", bufs=1))
        pt_pool = ctx.enter_context(tc.tile_pool(name="pt", bufs=6))
        rec_pool = ctx.enter_context(tc.tile_pool(name="rec", bufs=2))
        ysb_pool = ctx.enter_context(tc.tile_pool(name="ysb", bufs=8))
        ps_att = ctx.enter_context(tc.tile_pool(name="ps_att", bufs=2, space="PSUM"))
        ps_sc = ctx.enter_context(tc.tile_pool(name="ps_sc", bufs=2, space="PSUM"))
        ps_qw = ctx.enter_context(tc.tile_pool(name="ps_qw", bufs=2, space="PSUM"))

        # --- persistent tiles -------------------------------------------------
        # xt for s-block 0 and wkv are split into quarter tiles so compute can
        # start as soon as the first slivers land.
        xt0_t = [persist.tile([P, 4, 512], BF16, tag=f"xt0{h}", name=f"xt0{h}")
                 for h in range(4)]
        xt_t = [None] + [persist.tile([P, 16, 512], BF16, tag=f"xt{sb}", name=f"xt{sb}")
                         for sb in range(1, 4)]

        def xt_slice(sb, di, lo, hi):
            if sb == 0:
                return xt0_t[di // 4][:, di % 4, lo:hi]
            return xt_t[sb][:, di, lo:hi]

        wq_sb = persist.tile([P, 16, 512], BF16, tag="wq")
        wkv_t = [persist.tile([P, 4, 256], BF16, tag=f"wkv{h}", name=f"wkv{h}")
                 for h in range(4)]
        wo_sb = persist.tile([P, 4, 2048], BF16, tag="wo")
        qt_t = [[persist.tile([P, 512], BF16, tag=f"qt{r}_{sb}", name=f"qt{r}_{sb}")
                 for sb in range(4)] for r in range(4)]
        kt_t = [persist.tile([P, 512], BF16, tag=f"kt{sb}", name=f"kt{sb}")
                for sb in range(4)]
        v_t = [persist.tile([P, 4, 128], BF16, tag=f"v{sb}", name=f"v{sb}")
               for sb in range(4)]
        den_acc = [persist.tile([P, 1024], BF16, tag=f"den{r}", name=f"den{r}")
                   for r in range(4)]
        ones = persist.tile([P, P], BF16, tag="ones")

        # --- input DMAs (ordered: first-needed first) -------------------------
        xt_r = xt_d.rearrange("(t p) s -> p t s", p=P)
        wkv_r = wkv_d.rearrange("(t p) h -> p t h", p=P)
        for h in range(4):
            nc.sync.dma_start(out=wkv_t[h][:], in_=wkv_r[:, 4 * h:4 * h + 4, :])
            nc.sync.dma_start(out=xt0_t[h][:], in_=xt_r[:, 4 * h:4 * h + 4, 0:512])
        nc.sync.dma_start(out=wq_sb[:], in_=wq_d.rearrange("(t p) h -> p t h", p=P))
        for sb in range(1, 4):
            nc.sync.dma_start(out=xt_t[sb][:], in_=xt_r[:, :, sb * 512:(sb + 1) * 512])
        nc.sync.dma_start(out=wo_sb[:], in_=wo_d.rearrange("(r p) n -> p r n", p=P))

        nc.vector.memset(ones[:], 1.0)

        # warm the PE p-state while the first DMAs are in flight: the ramp to
        # full clock needs ~3us of continuous execution, so burn it on dummy
        # matmuls instead of the first real ones
        warm = ps_att.tile([P, 512], F32, tag="att")
        for i in range(80):
            nc.tensor.matmul(warm[:, 0:P], ones[:], ones[:],
                             start=(i == 0), stop=(i == 79))

        # --- work chunks ------------------------------------------------------
        def kt_chunk(sb):
            """K projection, pre-transposed, for a whole 512-wide s-block:
            KT [hd*2, s] comes straight out of the PE by using the K weights
            as the stationary operand -- no transpose pass at all."""
            ktp = ps_qw.tile([P, 512], F32, tag="qw")
            for di in range(16):
                nc.tensor.matmul(
                    ktp[:],
                    wkv_t[di // 4][:, di % 4, 0:128],
                    xt_slice(sb, di, 0, 512),
                    start=(di == 0),
                    stop=(di == 15),
                )
            nc.vector.tensor_copy(kt_t[sb][:], ktp[:])

        def v_chunk(sb, stl):
            """V projection (natural layout) for s-tile sb*4+stl."""
            vp = ps_qw.tile([P, 512], F32, tag="qw")
            for di in range(16):
                nc.tensor.matmul(
                    vp[:, 0:P],
                    xt_slice(sb, di, stl * P, (stl + 1) * P),
                    wkv_t[di // 4][:, di % 4, 128:256],
                    start=(di == 0),
                    stop=(di == 15),
                )
            nc.vector.tensor_copy(v_t[sb][:, stl, :], vp[:, 0:P])

        def qt_chunk(sb, r):
            """QT[r] for s-block sb."""
            qp = ps_qw.tile([P, 512], F32, tag="qw")
            for di in range(16):
                nc.tensor.matmul(
                    qp[:],
                    wq_sb[:, di, r * P:(r + 1) * P],
                    xt_slice(sb, di, 0, 512),
                    start=(di == 0),
                    stop=(di == 15),
                )
            nc.vector.tensor_copy(qt_t[r][sb][:], qp[:])

        ychunk_n = [0]

        def wo_chunk(st, nb, tail=False):
            """Output projection for s-tile st, output-column block nb."""
            if tail and ychunk_n[0] % 2 == 0:
                # the attention banks are dead at the tail: alternating pools
                # doubles the effective wp rotation depth
                wp = ps_att.tile([P, 512], F32, tag="att")
            else:
                wp = ps_qw.tile([P, 512], F32, tag="qw")
            ychunk_n[0] += 1
            stl = st % 4
            for r in range(4):
                nc.tensor.matmul(
                    wp[:],
                    qt_t[r][st // 4][:, stl * P:(stl + 1) * P],
                    wo_sb[:, r, nb * 512:(nb + 1) * 512],
                    start=(r == 0),
                    stop=(r == 3),
                )
            ysb = ysb_pool.tile([P, 512], F32, tag="y")
            # mid-stream evictions go to DVE (Act is exp-saturated there);
            # tail evictions go to Act, which is idle by then
            if tail:
                nc.scalar.copy(ysb[:], wp[:])
            else:
                nc.vector.tensor_copy(ysb[:], wp[:])
            nc.sync.dma_start(
                out=y_d[st * P:(st + 1) * P, nb * 512:(nb + 1) * 512], in_=ysb[:])

        # --- head: K/V for s-block 0 + QT for the first r-pair ----------------
        kt_chunk(0)
        for stl in range(4):
            v_chunk(0, stl)
        for r in range(2):
            qt_chunk(0, r)

        # fillers woven into the attention loop's PE slack.  urgent (KV/QT,
        # intra-attention deadlines) drain one per k0 slot; lazy (wo) one per
        # two slots so late passes stay PE-fed.  QT0 r2/r3 lead (needed by
        # qi0's second pass).
        urgent = deque()
        lazy = deque()
        for sb in range(1, 3):
            urgent.append(lambda sb=sb: kt_chunk(sb))
            for stl in range(4):
                urgent.append(lambda sb=sb, stl=stl: v_chunk(sb, stl))
        urgent.append(lambda: qt_chunk(0, 2))
        urgent.append(lambda: kt_chunk(3))
        for stl in range(4):
            urgent.append(lambda stl=stl: v_chunk(3, stl))
        urgent.append(lambda: qt_chunk(0, 3))
        for sb in range(1, 4):
            for r in range(4):
                urgent.append(lambda sb=sb, r=r: qt_chunk(sb, r))

        # --- attention --------------------------------------------------------
        for qi in range(4):
            for half in range(2):
                rpair = (2 * half, 2 * half + 1)
                last_pass = (qi == 3 and half == 1)
                att = {r: ps_att.tile([P, 512], F32, tag="att", name=f"att{qi}_{r}")
                       for r in rpair}

                def pv_and_den(k0, pk, skip_den=False):
                    for r, p in pk:
                        nc.tensor.matmul(
                            att[r][0:64, :],
                            v_t[k0 // 4][:, k0 % 4, 0:64],
                            p[:, 0:512],
                            start=(k0 == 0), stop=(k0 == 15),
                            tile_position=(0, 0), skip_group_check=True,
                        )
                        nc.tensor.matmul(
                            att[r][64:128, :],
                            v_t[k0 // 4][:, k0 % 4, 64:128],
                            p[:, 512:1024],
                            start=(k0 == 0), stop=(k0 == 15),
                            tile_position=(0, 64), skip_group_check=True,
                        )
                        if skip_den:
                            continue
                        if k0 == 0:
                            nc.vector.tensor_copy(den_acc[r][:], p[:])
                        else:
                            nc.vector.tensor_add(den_acc[r][:], den_acc[r][:], p[:])

                prev = None
                for k0 in range(16):
                    cur = []
                    for r in rpair:
                        sc = ps_sc.tile([P, 1024], F32, tag="sc")
                        nc.tensor.matmul(
                            sc[:, 0:512],
                            kt_t[k0 // 4][0:64, (k0 % 4) * P:(k0 % 4 + 1) * P],
                            qt_t[r][qi][0:64, :],
                            start=True, stop=True,
                            tile_position=(0, 0),
                        )
                        nc.tensor.matmul(
                            sc[:, 512:1024],
                            kt_t[k0 // 4][64:128, (k0 % 4) * P:(k0 % 4 + 1) * P],
                            qt_t[r][qi][64:128, :],
                            start=True, stop=True,
                            tile_position=(64, 0),
                        )
                        p = pt_pool.tile([P, 1024], BF16, tag="p")
                        nc.scalar.activation(p[:], sc[:], EXP, scale=SCALE)
                        cur.append((r, p))
                    if prev is not None:
                        pv_and_den(k0 - 1, prev)
                    if urgent:
                        urgent.popleft()()
                    elif lazy and k0 % 2 == 0 and (qi < 3 or len(lazy) > 8):
                        lazy.popleft()()
                    prev = cur
                # pass epilogue.  p(k0=15) stays out of the DVE den chain:
                # it is accumulated straight into den_h by a second
                # ones-matmul.  Emission order interleaves the den_acc
                # matmuls (no exp15 dependency) with the deferred PV(15) so
                # the PE never sits on the just-finished exp.
                p15 = dict(prev)
                # hybrid denominator: the first r of the pass (whose normalize
                # gates the next pass's att-bank rotation) takes the fast PE
                # ones-matmul path with the p15 shortcut; the second r (looser
                # deadline) reduces on the idle Pool engine instead.
                ra, rb = rpair
                if last_pass:
                    rb = None
                den_hs = {}
                fast_rs = [r for r in rpair if r != rb]
                for r in fast_rs:
                    for a in range(2):
                        den_h = ps_qw.tile([P, 512], F32, tag="qw",
                                           name=f"dall{qi}_{r}_{a}")
                        nc.tensor.matmul(den_h[:], ones[:],
                                         den_acc[r][:, a * 512:(a + 1) * 512],
                                         start=True, stop=False)
                        den_hs[(r, a)] = den_h
                pv_and_den(15, prev, skip_den=True)
                for r, p in prev:
                    if r == rb:
                        nc.vector.tensor_add(den_acc[rb][:], den_acc[rb][:], p[:])
                for r in fast_rs:
                    rec_a = rec_pool.tile([P, 1024], F32, tag="rec")
                    for a in range(2):
                        nc.tensor.matmul(den_hs[(r, a)][:], ones[:],
                                         p15[r][:, a * 512:(a + 1) * 512],
                                         start=False, stop=True)
                        nc.vector.reciprocal(rec_a[:, a * 512:(a + 1) * 512],
                                             den_hs[(r, a)][:])
                    nc.vector.tensor_mul(qt_t[r][qi][0:64, :], att[r][0:64, :],
                                         rec_a[0:64, 0:512])
                    nc.vector.tensor_mul(qt_t[r][qi][64:128, :], att[r][64:128, :],
                                         rec_a[64:128, 512:1024])
                if rb is not None:
                    rec_b = rec_pool.tile([P, 1024], F32, tag="rec")
                    den_rep = rec_pool.tile([P, 1024], F32, tag="drep")
                    nc.gpsimd.partition_all_reduce(
                        den_rep[:], den_acc[rb][:], channels=P,
                        reduce_op=bass_isa.ReduceOp.add)
                    nc.vector.reciprocal(rec_b[:], den_rep[:])
                    nc.vector.tensor_mul(qt_t[rb][qi][0:64, :], att[rb][0:64, :],
                                         rec_b[0:64, 0:512])
                    nc.vector.tensor_mul(qt_t[rb][qi][64:128, :], att[rb][64:128, :],
                                         rec_b[64:128, 512:1024])
                # bridge the epilogue chain with independent PE work so the
                # next pass (or the wo tail) never sees a cold PE; urgent
                # fillers (needed by the NEXT pass's first matmuls) go first
                for _ in range(8 if last_pass else 2):
                    if urgent:
                        urgent.popleft()()
                    elif lazy:
                        lazy.popleft()()
            # wo for this q-block becomes filler work for later q-blocks
            # (except the last q-block, which drains at the tail)
            if qi < 3:
                for nb in range(4):
                    for stl in range(4):
                        lazy.append(lambda st=qi * 4 + stl, nb=nb: wo_chunk(st, nb))

        while urgent:
            urgent.popleft()()
        while lazy:
            lazy.popleft()()
        for nb in range(4):
            for stl in range(4):
                wo_chunk(12 + stl, nb, tail=True)

    nc.compile()
    return nc


def make_in_maps(x, wq, wk, wv, wo):
    import ml_dtypes
    BF = ml_dtypes.bfloat16

    x = np.asarray(x, dtype=np.float32)
    wq = np.asarray(wq, dtype=np.float32)
    wk = np.asarray(wk, dtype=np.float32)
    wv = np.asarray(wv, dtype=np.float32)
    wo = np.asarray(wo, dtype=np.float32)
    in_maps = []
    for c in range(8):
        b, t = divmod(c, 4)
        g0 = 2 * t
        perm = np.array(
            [(4 * (g0 + a) + r) * 64 + j
             for r in range(4) for a in range(2) for j in range(64)],
            dtype=np.int64,
        )
        in_maps.append({
            "xt": np.ascontiguousarray(x[b].T).astype(BF),
            "wq": np.ascontiguousarray(wq[:, perm]).astype(BF),
            "wkv": np.ascontiguousarray(np.concatenate(
                [wk[:, g0 * 64:(g0 + 2) * 64], wv[:, g0 * 64:(g0 + 2) * 64]],
                axis=1)).astype(BF),
            "wo": np.ascontiguousarray(wo[perm, :]).astype(BF),
        })
    return in_maps


def kernel(x, wq, wk, wv, wo):
    from concourse.bass_utils import run_bass_kernel_spmd

    if "nc" not in _CACHE:
        _CACHE["nc"] = build_nc()
    nc = _CACHE["nc"]
    in_maps = make_in_maps(x, wq, wk, wv, wo)
    results = run_bass_kernel_spmd(nc, in_maps, list(range(8))).results
    y = np.empty((BSZ, SEQ, DIM), np.float32)
    for b in range(2):
        y[b] = (results[4 * b]["y"] + results[4 * b + 1]["y"]
                + results[4 * b + 2]["y"] + results[4 * b + 3]["y"])
    return y
